# revision 4
# baseline (speedup 1.0000x reference)
"""Trainium2 Bass kernel for nn_AtNeuron_18622978195626.

Temporal diff-coding scan over T=8 steps of batched 512x512x512 matmuls:
    inputs x, y: [(T+1)*B, 512, 512] = [9, 8, 512, 512], out[0] = 0
    carries xv_t = sum_{s<=t} x_s/s,  yv_t = sum_{s<=t} y_s/s
    reference step:  out_t = x_t@y_t/t + x_t@yv_{t-1} + xv_{t-1}@y_t

Telescoping identity (exact): with U_t = xv_t @ yv_t,
    U_t - U_{t-1} = (x_t@yv_{t-1} + xv_{t-1}@y_t + x_t@y_t/t) / t = out_t / t
so   out_t = t*U_t - t*U_{t-1}.
One 512^3 matmul per step (16 PE matmuls) instead of the reference's three:
the PE does 128 matmuls total and the kernel is HBM-bandwidth-bound.

Per step: chunk-wise carry updates on DVE, U_t on the PE, S_t = t*U_t as a
scaled PSUM->SBUF copy on ACT, out_t = S_t - (t/(t-1))*S_{t-1} on GpSimd,
store on ACT's HWDGE ring (loads ride Sync's ring).

Sharding: batch dim B=8, one batch element per NeuronCore (data parallel, no
communication). x is transposed on the host during sharding so it lands in
SBUF partition-on-k ([K, M]) as the PE's stationary operand requires; y's
natural layout [K, N] already suits the moving operand. Matmuls run in
float32r (full-rate fp32 path; rel err ~1e-3, well inside the 2e-2 gate).
"""

import sys

if "/opt/trn_rl_repo" not in sys.path:
    sys.path.insert(0, "/opt/trn_rl_repo")

import numpy as np

import concourse.mybir as mybir
import concourse.tile as tile
from concourse import bacc
from concourse.bass_utils import run_bass_kernel_spmd

T = 8          # scan steps (t = 1..8); t=0 output is identically zero
B = 8          # batch = number of cores
D = 512        # matrix dim
P = 128        # partitions
KO = D // P    # k/m outer tiles = 4

MM_DT = mybir.dt.float32r   # full-rate fp32 matmul path
F32 = mybir.dt.float32

_CACHE = {}


def _build():
    """Build + compile the single-core program (same program on all 8 cores)."""
    if "nc" in _CACHE:
        return _CACHE["nc"]

    nc = bacc.Bacc("TRN2", target_bir_lowering=False, debug=False)
    # xT[t] is x_{t+1}.T, layout [K, M]; y[t] is y_{t+1}, layout [K, N]
    xT_d = nc.dram_tensor("xT", [T, D, D], MM_DT, kind="ExternalInput").ap()
    y_d = nc.dram_tensor("y", [T, D, D], MM_DT, kind="ExternalInput").ap()
    o_d = nc.dram_tensor("out", [T, D, D], F32, kind="ExternalOutput").ap()

    with tile.TileContext(nc) as tc:
        with (
            tc.tile_pool(name="xin", bufs=4 * KO) as xpool,
            tc.tile_pool(name="yin", bufs=4 * KO) as ypool,
            tc.tile_pool(name="yvp", bufs=2 * KO) as yvpool,
            tc.tile_pool(name="xvp", bufs=2 * KO) as xvpool,
            tc.tile_pool(name="spool", bufs=2) as spool,
            tc.tile_pool(name="outs", bufs=3) as opool,
            tc.tile_pool(name="psum", bufs=2, space="PSUM") as pspool,
        ):
            # Chunked loads in step order on nc.sync's FIFO HWDGE ring; each
            # chunk is a contiguous 256 KB block of DRAM.
            xch = [[None] * KO for _ in range(T)]
            ych = [[None] * KO for _ in range(T)]
            for t in range(T):
                for k in range(KO):
                    xc = xpool.tile([P, D], MM_DT, tag="xT")
                    nc.sync.dma_start(xc[:], xT_d[t, k * P:(k + 1) * P, :])
                    xch[t][k] = xc
                    yc = ypool.tile([P, D], MM_DT, tag="y")
                    nc.sync.dma_start(yc[:], y_d[t, k * P:(k + 1) * P, :])
                    ych[t][k] = yc

            yv = ych[0]   # yv_1 = y_1, xv_1 = x_1 (inv = 1)
            xvT = xch[0]
            S_prev = None
            for s in range(T):
                t_step = s + 1
                inv = 1.0 / t_step
                if s > 0:
                    # per-chunk carry updates into fresh tiles (pipeline ahead)
                    yv_new, xv_new = [], []
                    for k in range(KO):
                        cy = yvpool.tile([P, D], MM_DT, tag="yv")
                        nc.vector.scalar_tensor_tensor(
                            cy[:], ych[s][k][:], inv, yv[k][:],
                            mybir.AluOpType.mult, mybir.AluOpType.add,
                        )
                        yv_new.append(cy)
                        cx = xvpool.tile([P, D], MM_DT, tag="xvT")
                        nc.vector.scalar_tensor_tensor(
                            cx[:], xch[s][k][:], inv, xvT[k][:],
                            mybir.AluOpType.mult, mybir.AluOpType.add,
                        )
                        xv_new.append(cx)
                    yv, xvT = yv_new, xv_new

                # U_t = xv_t @ yv_t
                ps = pspool.tile([P, KO, D], F32, tag="ps")
                for mo in range(KO):
                    for k in range(KO):
                        nc.tensor.matmul(
                            ps[:, mo, :], xvT[k][:, mo * P:(mo + 1) * P], yv[k][:],
                            start=(k == 0), stop=(k == KO - 1),
                        )

                if s == 0:
                    # out_1 = 1*U_1; the staged output doubles as S_1
                    S_t = opool.tile([P, KO, D], F32, tag="out")
                    nc.scalar.mul(S_t[:], ps[:], 1.0)
                    out_t = S_t
                else:
                    # S_t = t*U_t (scaled PSUM drain on ACT)
                    S_t = spool.tile([P, KO, D], F32, tag="S")
                    nc.scalar.mul(S_t[:], ps[:], float(t_step))
                    # out_t = S_t - (t/(t-1))*S_{t-1}
                    out_t = opool.tile([P, KO, D], F32, tag="out")
                    c = -float(t_step) / float(t_step - 1)
                    nc.vector.scalar_tensor_tensor(
                        out_t[:], S_prev[:], c, S_t[:],
                        mybir.AluOpType.mult, mybir.AluOpType.add,
                    )
                S_prev = S_t

                nc.scalar.dma_start(
                    o_d[s].rearrange("(mo mi) n -> mi mo n", mi=P), out_t[:]
                )

    nc.compile()
    _CACHE["nc"] = nc
    return nc


def _run(inputs, trace=False):
    x = np.ascontiguousarray(np.asarray(inputs["x"], dtype=np.float32))
    y = np.ascontiguousarray(np.asarray(inputs["y"], dtype=np.float32))
    x5 = x.reshape(T + 1, B, D, D)
    y5 = y.reshape(T + 1, B, D, D)

    in_maps = []
    for c in range(B):
        in_maps.append({
            "xT": np.ascontiguousarray(x5[1:, c].transpose(0, 2, 1)),
            "y": np.ascontiguousarray(y5[1:, c]),
        })

    nc = _build()
    res = run_bass_kernel_spmd(nc, in_maps, core_ids=list(range(B)), trace=trace)

    out = np.zeros((T + 1, B, D, D), dtype=np.float32)
    for c in range(B):
        out[1:, c] = res.results[c]["out"]
    return out.reshape((T + 1) * B, D, D), res


def kernel(**inputs) -> np.ndarray:
    out, _ = _run(inputs, trace=False)
    return out


def kernel_traced(inputs):
    """Like kernel() but with NTFF profiling; returns (out, BassKernelResults)."""
    return _run(inputs, trace=True)
